# revision 32
# baseline (speedup 1.0000x reference)
"""Trainium2 Bass kernel for CenterLoss (image-centre loss + class-centre loss).

Math (reference):
  img   = mean_b ||x_b - centers[labels_b]||^2
  c     = centers[labels]                       # [B, D]
  n_i   = ||c_i||^2
  pd    = (n_i + n_j - 2 c_i.c_j) / D           # [B, B]
  same  = labels_2[i] == labels_2[j]
  intra = sum_{same} pd / n_same
  inter = sum_{!same} 1/(1+pd) / n_diff
  out   = img + intra + inter                   # (img only when y == 1)

Device strategy (8 cores, data-parallel over batch rows):
  * Each core gathers all B center rows (indirect DMA), transposes them on the
    PE into A[kc] = c^T tiles ([D-part, B-free]) for use as the matmul moving
    operand.
  * The pairwise term is ONE augmented matmul per (row-block, col-chunk):
    extra contraction rows carry [n/D, ones, sqrt(M)*onehot(labels_2)] so PSUM
    directly holds  1 + pd + M*same_mask  with M = 2^30.  The reciprocal then
    maps same-pairs to ~1/M ~ 0, eliminating all masking work.
  * intra is computed analytically from group sums:
      intra_sum = sum_g (2*cnt_g*sum_{i in g} n_i - 2*||s_g||^2)/D,
    with s_g (partial, own rows) computed on-device by a tiny one-hot matmul
    and combined on host.
  * Each core returns partial row-sums; host does the final tiny reductions.
"""

import numpy as np

# Problem constants (hardcoded per harness contract).
B = 4096
D = 512
NCLS = 10000
NG = 50
NCORES = 8

_DEFAULT_CFG = dict(B=B, D=D, NCLS=NCLS, NG=NG, NCORES=NCORES)

_cache = {}
_last_results = None


def _import_concourse():
    try:
        import concourse.bass  # noqa: F401
    except ImportError:
        import sys

        sys.path.insert(0, "/opt/trn_rl_repo")


def _split_sync_waits(module_dict, max_waits=1):
    """The walrus build in this container accepts at most one sync-wait per
    instruction; Tile emits several.  Hoist excess waits onto NoOps inserted
    just before the instruction on the same engine (engine streams are
    serial, so waiting earlier is equivalent)."""
    counter = [0]
    for f in module_dict["functions"]:
        for b in f["blocks"]:
            out = []
            for inst in b["instructions"]:
                si = inst.get("sync_info")
                waits = (si or {}).get("on_wait") or []
                if len(waits) > max_waits:
                    keep = waits[-max_waits:]
                    extra = waits[:-max_waits]
                    for i in range(0, len(extra), max_waits):
                        counter[0] += 1
                        out.append(
                            {
                                "debug": inst.get("debug", 0),
                                "engine": inst["engine"],
                                "ins": [],
                                "name": f"ws{counter[0]}_{inst['name']}",
                                "opcode": "NoOp",
                                "outs": [],
                                "sync_info": {
                                    "on_update": [],
                                    "on_wait": extra[i : i + max_waits],
                                },
                                "text_hint": "waitsplit",
                            }
                        )
                    si["on_wait"] = keep
                out.append(inst)
            b["instructions"] = out
    return module_dict


def build_program(cfg=None, stages=(1,2,3,4), s3_mode=3):
    """Build the (SPMD-uniform) Bass program. Returns the Bass object."""
    _import_concourse()
    from contextlib import ExitStack

    import concourse.bass as bass
    import concourse.tile as tile
    from concourse import mybir

    cfg = dict(_DEFAULT_CFG if cfg is None else cfg)
    cB, cD, cNCLS, cNG, cNC = cfg["B"], cfg["D"], cfg["NCLS"], cfg["NG"], cfg["NCORES"]
    ROWS = cB // cNC          # rows owned per core
    RBLK = ROWS // 128        # 128-row blocks per core
    NCH = cB // 512           # 512-wide column chunks of the pairwise matrix
    KC = cD // 128            # 128-row contraction chunks of D
    NRC = cB // 128           # 128-row gather chunks (all rows)
    GB = 1                    # rows per partition per indirect DMA (multi-index gather is broken on this walrus)
    KE = 128                  # extra (augmented) contraction rows
    KO = 64                   # one-hot column pad for s_g matmul
    SQM = 32768.0             # sqrt(M), M = 2^30

    f32 = mybir.dt.float32
    f32r = mybir.dt.float32r
    i32 = mybir.dt.int32
    OP = mybir.AluOpType
    AF = mybir.ActivationFunctionType
    AX = mybir.AxisListType

    nc = bass.Bass("TRN2", target_bir_lowering=False, debug=False)

    centers = nc.dram_tensor("centers", [cNCLS, cD], f32, kind="ExternalInput").ap()
    lab = nc.dram_tensor("lab", [cB], i32, kind="ExternalInput").ap()
    labo = nc.dram_tensor("labo", [ROWS], i32, kind="ExternalInput").ap()
    xs = nc.dram_tensor("xs", [ROWS, cD], f32, kind="ExternalInput").ap()
    eb = nc.dram_tensor("eb", [KE, cB], f32r, kind="ExternalInput").ap()
    lb = nc.dram_tensor("lb", [KE, ROWS], f32r, kind="ExternalInput").ap()
    orow = nc.dram_tensor("orow", [ROWS, KO], f32, kind="ExternalInput").ap()
    onesv = nc.dram_tensor("onesv", [128, 1], f32r, kind="ExternalInput").ap()
    idv = nc.dram_tensor("idv", [128, 128], f32, kind="ExternalInput").ap()

    racc_d = nc.dram_tensor("racc", [128, RBLK * NCH], f32, kind="ExternalOutput").ap()
    imgacc_d = nc.dram_tensor("imgacc", [128, RBLK], f32, kind="ExternalOutput").ap()
    sgout_d = nc.dram_tensor("sgout", [KO, cD], f32, kind="ExternalOutput").ap()
    nout_d = nc.dram_tensor("nout", [1, cB], f32r, kind="ExternalOutput").ap()


    with tile.TileContext(nc) as tc, ExitStack() as ctx:
        constp = ctx.enter_context(tc.tile_pool(name="const", bufs=1))
        apool = ctx.enter_context(tc.tile_pool(name="amat", bufs=1))
        psg = ctx.enter_context(tc.tile_pool(name="psg", bufs=1, space="PSUM"))

        identity = constp.tile([128, 128], f32, tag="ident")
        nc.sync.dma_start(identity[:], idv[:])
        ones_col = constp.tile([128, 1], f32r, tag="ones")
        nc.sync.dma_start(ones_col[:], onesv[:])

        lab_sb = constp.tile([128, NRC], i32, tag="lab")
        nc.sync.dma_start(lab_sb[:], lab.rearrange("(c p) -> p c", p=128))
        labo_sb = constp.tile([128, RBLK], i32, tag="labo")
        nc.sync.dma_start(labo_sb[:], labo.rearrange("(c p) -> p c", p=128))

        E = constp.tile([KE, cB], f32r, tag="E")
        nc.sync.dma_start(E[:], eb[:])
        Le = constp.tile([KE, ROWS], f32r, tag="Le")
        nc.sync.dma_start(Le[:], lb[:])

        racc = constp.tile([128, RBLK * NCH], f32, tag="racc")
        imgacc = constp.tile([128, RBLK], f32, tag="imgacc")

        A = [apool.tile([128, cB], f32r, tag=f"A{kc}", name=f"A{kc}") for kc in range(KC)]
        Lc = [
            [apool.tile([128, 128], f32r, tag=f"Lc{rb}_{kc}", name=f"Lc{rb}_{kc}") for kc in range(KC)]
            for rb in range(RBLK)
        ]

        sg_ps = psg.tile([KO, cD], f32, space="PSUM", tag="sg", name="sg_ps") if 2 in stages else None

        # ---- S2: own-row pipeline (gathers own rows; builds Lc, Le, sg, img)
        with tc.tile_pool(name="s2", bufs=2) as s2p, tc.tile_pool(
            name="s2ps", bufs=2, space="PSUM"
        ) as s2ps:
            for rb in (range(RBLK) if 2 in stages else []):
                crow_o = s2p.tile([128, cD], f32, tag="crow_o")
                nc.gpsimd.indirect_dma_start(
                    out=crow_o[:],
                    out_offset=None,
                    in_=centers[:],
                    in_offset=bass.IndirectOffsetOnAxis(
                        ap=labo_sb[:, rb : rb + 1], axis=0
                    ),
                )
                xsb = s2p.tile([128, cD], f32, tag="xsb")
                nc.sync.dma_start(xsb[:], xs[rb * 128 : (rb + 1) * 128, :])

                # s_g partial: one-hot^T @ c over own rows (accumulated)
                orow_sb = s2p.tile([128, KO], f32, tag="orow")
                nc.sync.dma_start(orow_sb[:], orow[rb * 128 : (rb + 1) * 128, :])
                nc.tensor.matmul(
                    out=sg_ps[:],
                    lhsT=orow_sb[:],
                    rhs=crow_o[:],
                    start=(rb == 0),
                    stop=(rb == RBLK - 1),
                )

                # image loss partial: sum_d (x - c)^2 per own row
                diff = s2p.tile([128, cD], f32, tag="diff")
                nc.vector.tensor_tensor(
                    out=diff[:], in0=xsb[:], in1=crow_o[:], op=OP.subtract
                )
                sqv = s2p.tile([128, cD], f32, tag="sqv")
                nc.vector.tensor_tensor(
                    out=sqv[:], in0=diff[:], in1=diff[:], op=OP.mult
                )
                nc.vector.tensor_reduce(
                    out=imgacc[:, rb : rb + 1], in_=sqv[:], axis=AX.X, op=OP.add
                )

                # n for own rows -> free layout via [128,1] PE transpose
                csq = s2p.tile([128, cD], f32, tag="csq")
                nc.vector.tensor_tensor(
                    out=csq[:], in0=crow_o[:], in1=crow_o[:], op=OP.mult
                )
                ncol = s2p.tile([128, 1], f32, tag="ncol")
                nc.vector.tensor_reduce(
                    out=ncol[:], in_=csq[:], axis=AX.X, op=OP.add
                )
                ntp = s2ps.tile([1, 128], f32, space="PSUM", tag="tp2", name="ntp")
                nc.tensor.transpose(out=ntp[:], in_=ncol[:], identity=identity[:])
                nc.scalar.activation(
                    out=Le[64:65, rb * 128 : (rb + 1) * 128],
                    in_=ntp[0:1, :],
                    func=AF.Copy,
                    scale=1.0 / cD,
                    bias=1.0,
                )

                # Lc[rb][kc] = -2 c_own^T / D: transpose then scaled copy
                for kc in range(KC):
                    tp = s2ps.tile([128, 128], f32, space="PSUM", tag="tp2")
                    nc.tensor.transpose(
                        out=tp[:],
                        in_=crow_o[:, kc * 128 : (kc + 1) * 128],
                        identity=identity[:],
                    )
                    nc.scalar.activation(
                        out=Lc[rb][kc][:], in_=tp[:], func=AF.Copy, scale=-2.0 / cD
                    )

        # ---- fused column-chunk pipeline: gather -> transpose -> n -> pairwise
        inv_sqrt_d = 1.0 / float(np.sqrt(cD))
        with tc.tile_pool(name="f1", bufs=6) as s1p, tc.tile_pool(
            name="f1ps", bufs=2, space="PSUM"
        ) as s1ps, tc.tile_pool(name="f3", bufs=2) as s3p, tc.tile_pool(
            name="f3ps", bufs=2, space="PSUM"
        ) as s3ps, tc.tile_pool(name="f4", bufs=3) as s4p, tc.tile_pool(
            name="f4ps", bufs=3, space="PSUM"
        ) as s4ps:
            for ch in range(NCH):
                for c in range(512 // 128):
                    rc = ch * 4 + c
                    crow = s1p.tile([128, cD], f32, tag="crow")
                    nc.gpsimd.indirect_dma_start(
                        out=crow[:],
                        out_offset=None,
                        in_=centers[:],
                        in_offset=bass.IndirectOffsetOnAxis(
                            ap=lab_sb[:, rc : rc + 1], axis=0
                        ),
                    )
                    for kc in range(KC):
                        tp = s1ps.tile([128, 128], f32, space="PSUM", tag="tp")
                        nc.tensor.transpose(
                            out=tp[:],
                            in_=crow[:, kc * 128 : (kc + 1) * 128],
                            identity=identity[:],
                        )
                        nc.vector.tensor_copy(
                            out=A[kc][:, rc * 128 : (rc + 1) * 128], in_=tp[:]
                        )

                # n for this column chunk: ones^T @ (A/sqrt(D))^2
                np_t = s3ps.tile([1, 512], f32, space="PSUM", tag="npt")
                for kc in range(KC):
                    a2 = s3p.tile([128, 512], f32r, tag=f"a2_{kc}", name=f"a2_{kc}_{ch}")
                    nc.scalar.activation(
                        out=a2[:],
                        in_=A[kc][:, ch * 512 : (ch + 1) * 512],
                        func=AF.Square,
                        scale=inv_sqrt_d,
                    )
                    nc.tensor.matmul(
                        out=np_t[:],
                        lhsT=ones_col[:],
                        rhs=a2[:],
                        start=(kc == 0),
                        stop=(kc == KC - 1),
                    )
                nc.scalar.activation(
                    out=E[0:1, ch * 512 : (ch + 1) * 512], in_=np_t[:], func=AF.Copy
                )

                # pairwise: PSUM = 1 + pd + M*mask for (rb, ch); sum of 1/(.)
                for rb in range(RBLK):
                    pd_ps = s4ps.tile([128, 512], f32, space="PSUM", tag="pd")
                    for kc in range(KC):
                        nc.tensor.matmul(
                            out=pd_ps[:],
                            lhsT=Lc[rb][kc][:],
                            rhs=A[kc][:, ch * 512 : (ch + 1) * 512],
                            start=(kc == 0),
                            stop=False,
                        )
                    nc.tensor.matmul(
                        out=pd_ps[:],
                        lhsT=Le[:, rb * 128 : (rb + 1) * 128],
                        rhs=E[:, ch * 512 : (ch + 1) * 512],
                        start=False,
                        stop=True,
                    )
                    lnv = s4p.tile([128, 512], f32, tag="lnv")
                    nc.scalar.activation(out=lnv[:], in_=pd_ps[:], func=AF.Ln)
                    rdum = s4p.tile([128, 512], f32, tag="rdum")
                    nc.scalar.activation(
                        out=rdum[:],
                        in_=lnv[:],
                        func=AF.Exp,
                        scale=-1.0,
                        accum_out=racc[:, rb * NCH + ch : rb * NCH + ch + 1],
                    )

        # ---- S5: outputs
        if 2 in stages:
            sg_sb = constp.tile([KO, cD], f32, tag="sgsb")
            nc.vector.tensor_copy(out=sg_sb[:], in_=sg_ps[:])
            nc.sync.dma_start(sgout_d[:], sg_sb[:])
            nc.sync.dma_start(imgacc_d[:], imgacc[:])
        nc.sync.dma_start(racc_d[:], racc[:])
        nc.sync.dma_start(nout_d[:], E[0:1, :])

    import json as _json

    _orig_tjb = nc.to_json_bytes

    def _patched_tjb():
        m = _json.loads(_orig_tjb())
        _split_sync_waits(m)
        return _json.dumps(m).encode()

    nc.to_json_bytes = _patched_tjb
    return nc


def make_inputs(x, lab_i32, l2, cfg=None):
    """Host-side per-core input maps. l2 is int array [B] of labels_2."""
    cfg = dict(_DEFAULT_CFG if cfg is None else cfg)
    cB, cNG, cNC = cfg["B"], cfg["NG"], cfg["NCORES"]
    ROWS = cB // cNC
    KE = 128
    KO = 64
    SQM = 32768.0

    # E side: row0 = n/D (device), rows 1..NG = sqrt(M)*onehot, row64 = ones
    eb = np.zeros((KE, cB), np.float32)
    eb[64, :] = 1.0
    eb[1 + l2, np.arange(cB)] = SQM

    in_maps = []
    for k in range(cNC):
        sl = slice(k * ROWS, (k + 1) * ROWS)
        l2o = l2[sl]
        # L side: row0 = ones, rows 1..NG = sqrt(M)*onehot, row64 = n/D+1 (device)
        lbm = np.zeros((KE, ROWS), np.float32)
        lbm[0, :] = 1.0
        lbm[1 + l2o, np.arange(ROWS)] = SQM
        orow = np.zeros((ROWS, KO), np.float32)
        orow[np.arange(ROWS), l2o] = 1.0
        in_maps.append(
            {
                "lab": lab_i32,
                "onesv": np.ones((128, 1), np.float32),
                "idv": np.eye(128, dtype=np.float32),
                "labo": np.ascontiguousarray(lab_i32[sl]),
                "xs": np.ascontiguousarray(x[sl]),
                "eb": eb,
                "lb": lbm,
                "orow": orow,
            }
        )
    return in_maps


def combine(results, l2, yv, cfg=None):
    """Host-side combination of per-core partial outputs -> scalar loss."""
    cfg = dict(_DEFAULT_CFG if cfg is None else cfg)
    cB, cD, cNG = cfg["B"], cfg["D"], cfg["NG"]

    img = sum(r["imgacc"].astype(np.float64).sum() for r in results) / cB
    if yv == 1:
        return np.float32(img)

    n = results[0]["nout"][0].astype(np.float64) * cD  # nout holds n/D
    sg = sum(r["sgout"][:cNG].astype(np.float64) for r in results)
    inter_sum = sum(r["racc"].astype(np.float64).sum() for r in results)

    cnt = np.bincount(l2, minlength=cNG).astype(np.float64)
    nsum = np.bincount(l2, weights=n, minlength=cNG)
    n_same = float((cnt**2).sum())
    n_diff = float(cB * cB - n_same)
    intra_sum = float(((2.0 * cnt * nsum - 2.0 * (sg * sg).sum(axis=1)) / cD).sum())
    intra = intra_sum / max(n_same, 1.0)
    inter = float(inter_sum) / max(n_diff, 1.0)
    return np.float32(img + intra + inter)


def kernel(x, labels, labels_2, y, centers):
    global _last_results
    _import_concourse()
    from concourse.bass_utils import run_bass_kernel_spmd

    x = np.ascontiguousarray(np.asarray(x, dtype=np.float32))
    centers = np.ascontiguousarray(np.asarray(centers, dtype=np.float32))
    lab_i32 = np.ascontiguousarray(np.asarray(labels).astype(np.int32))
    l2 = np.asarray(labels_2).astype(np.int64)
    yv = int(np.asarray(y))

    if "prog" not in _cache:
        _cache["prog"] = build_program()
    nc = _cache["prog"]

    in_maps = make_inputs(x, lab_i32, l2)
    for m in in_maps:
        m["centers"] = centers

    res = run_bass_kernel_spmd(nc, in_maps, list(range(NCORES)))
    _last_results = res
    return combine(res.results, l2, yv)


# revision 34
# speedup vs baseline: 1.0883x; 1.0883x over previous
"""Trainium2 Bass kernel for CenterLoss (image-centre loss + class-centre loss).

Math (reference):
  img   = mean_b ||x_b - centers[labels_b]||^2
  c     = centers[labels]                       # [B, D]
  n_i   = ||c_i||^2
  pd    = (n_i + n_j - 2 c_i.c_j) / D           # [B, B]
  same  = labels_2[i] == labels_2[j]
  intra = sum_{same} pd / n_same
  inter = sum_{!same} 1/(1+pd) / n_diff
  out   = img + intra + inter                   # (img only when y == 1)

Device strategy (8 cores, data-parallel over batch rows):
  * Each core gathers all B center rows (indirect DMA), transposes them on the
    PE into A[kc] = c^T tiles ([D-part, B-free]) for use as the matmul moving
    operand.
  * The pairwise term is ONE augmented matmul per (row-block, col-chunk):
    extra contraction rows carry [n/D, ones, sqrt(M)*onehot(labels_2)] so PSUM
    directly holds  1 + pd + M*same_mask  with M = 2^30.  The reciprocal then
    maps same-pairs to ~1/M ~ 0, eliminating all masking work.
  * intra is computed analytically from group sums:
      intra_sum = sum_g (2*cnt_g*sum_{i in g} n_i - 2*||s_g||^2)/D,
    with s_g (partial, own rows) computed on-device by a tiny one-hot matmul
    and combined on host.
  * Each core returns partial row-sums; host does the final tiny reductions.
"""

import numpy as np

# Problem constants (hardcoded per harness contract).
B = 4096
D = 512
NCLS = 10000
NG = 50
NCORES = 8

_DEFAULT_CFG = dict(B=B, D=D, NCLS=NCLS, NG=NG, NCORES=NCORES)

_cache = {}
_last_results = None


def _import_concourse():
    try:
        import concourse.bass  # noqa: F401
    except ImportError:
        import sys

        sys.path.insert(0, "/opt/trn_rl_repo")


def _split_sync_waits(module_dict, max_waits=1):
    """The walrus build in this container accepts at most one sync-wait per
    instruction; Tile emits several.  Hoist excess waits onto NoOps inserted
    just before the instruction on the same engine (engine streams are
    serial, so waiting earlier is equivalent)."""
    counter = [0]
    for f in module_dict["functions"]:
        for b in f["blocks"]:
            out = []
            for inst in b["instructions"]:
                si = inst.get("sync_info")
                waits = (si or {}).get("on_wait") or []
                if len(waits) > max_waits:
                    keep = waits[-max_waits:]
                    extra = waits[:-max_waits]
                    for i in range(0, len(extra), max_waits):
                        counter[0] += 1
                        out.append(
                            {
                                "debug": inst.get("debug", 0),
                                "engine": inst["engine"],
                                "ins": [],
                                "name": f"ws{counter[0]}_{inst['name']}",
                                "opcode": "NoOp",
                                "outs": [],
                                "sync_info": {
                                    "on_update": [],
                                    "on_wait": extra[i : i + max_waits],
                                },
                                "text_hint": "waitsplit",
                            }
                        )
                    si["on_wait"] = keep
                out.append(inst)
            b["instructions"] = out
    return module_dict


def build_program(cfg=None, stages=(1,2,3,4), s3_mode=3):
    """Build the (SPMD-uniform) Bass program. Returns the Bass object."""
    _import_concourse()
    from contextlib import ExitStack

    import concourse.bass as bass
    import concourse.tile as tile
    from concourse import mybir

    cfg = dict(_DEFAULT_CFG if cfg is None else cfg)
    cB, cD, cNCLS, cNG, cNC = cfg["B"], cfg["D"], cfg["NCLS"], cfg["NG"], cfg["NCORES"]
    ROWS = cB // cNC          # rows owned per core
    RBLK = ROWS // 128        # 128-row blocks per core
    NCH = cB // 512           # 512-wide column chunks of the pairwise matrix
    KC = cD // 128            # 128-row contraction chunks of D
    NRC = cB // 128           # 128-row gather chunks (all rows)
    GB = 1                    # rows per partition per indirect DMA (multi-index gather is broken on this walrus)
    KE = 128                  # extra (augmented) contraction rows
    KO = 64                   # one-hot column pad for s_g matmul
    SQM = 32768.0             # sqrt(M), M = 2^30

    f32 = mybir.dt.float32
    f32r = mybir.dt.float32r
    i32 = mybir.dt.int32
    OP = mybir.AluOpType
    AF = mybir.ActivationFunctionType
    AX = mybir.AxisListType

    nc = bass.Bass("TRN2", target_bir_lowering=False, debug=False)

    centers = nc.dram_tensor("centers", [cNCLS, cD], f32, kind="ExternalInput").ap()
    lab = nc.dram_tensor("lab", [cB], i32, kind="ExternalInput").ap()
    labo = nc.dram_tensor("labo", [ROWS], i32, kind="ExternalInput").ap()
    xs = nc.dram_tensor("xs", [ROWS, cD], f32, kind="ExternalInput").ap()
    eb = nc.dram_tensor("eb", [KE, cB], f32r, kind="ExternalInput").ap()
    lb = nc.dram_tensor("lb", [KE, ROWS], f32r, kind="ExternalInput").ap()
    orow = nc.dram_tensor("orow", [ROWS, KO], f32, kind="ExternalInput").ap()
    onesv = nc.dram_tensor("onesv", [128, 1], f32r, kind="ExternalInput").ap()
    idv = nc.dram_tensor("idv", [128, 128], f32, kind="ExternalInput").ap()

    racc_d = nc.dram_tensor("racc", [128, RBLK * NCH], f32, kind="ExternalOutput").ap()
    imgacc_d = nc.dram_tensor("imgacc", [128, RBLK], f32, kind="ExternalOutput").ap()
    sgout_d = nc.dram_tensor("sgout", [KO, cD], f32, kind="ExternalOutput").ap()
    nout_d = nc.dram_tensor("nout", [1, cB], f32r, kind="ExternalOutput").ap()


    with tile.TileContext(nc) as tc, ExitStack() as ctx:
        constp = ctx.enter_context(tc.tile_pool(name="const", bufs=1))
        apool = ctx.enter_context(tc.tile_pool(name="amat", bufs=1))
        psg = ctx.enter_context(tc.tile_pool(name="psg", bufs=1, space="PSUM"))

        identity = constp.tile([128, 128], f32, tag="ident")
        nc.sync.dma_start(identity[:], idv[:])
        ones_col = constp.tile([128, 1], f32r, tag="ones")
        nc.sync.dma_start(ones_col[:], onesv[:])

        lab_sb = constp.tile([128, NRC], i32, tag="lab")
        nc.sync.dma_start(lab_sb[:], lab.rearrange("(c p) -> p c", p=128))
        labo_sb = constp.tile([128, RBLK], i32, tag="labo")
        nc.sync.dma_start(labo_sb[:], labo.rearrange("(c p) -> p c", p=128))

        E = constp.tile([KE, cB], f32r, tag="E")
        nc.sync.dma_start(E[:], eb[:])
        Le = constp.tile([KE, ROWS], f32r, tag="Le")
        nc.sync.dma_start(Le[:], lb[:])

        racc = constp.tile([128, RBLK * NCH], f32, tag="racc")
        imgacc = constp.tile([128, RBLK], f32, tag="imgacc")

        A = [apool.tile([128, cB], f32r, tag=f"A{kc}", name=f"A{kc}") for kc in range(KC)]
        Lc = [
            [apool.tile([128, 128], f32r, tag=f"Lc{rb}_{kc}", name=f"Lc{rb}_{kc}") for kc in range(KC)]
            for rb in range(RBLK)
        ]

        sg_ps = psg.tile([KO, cD], f32, space="PSUM", tag="sg", name="sg_ps") if 2 in stages else None

        # ---- S2: own-row pipeline (gathers own rows; builds Lc, Le, sg, img)
        s2p = ctx.enter_context(tc.tile_pool(name="s2", bufs=2))
        s2ps = ctx.enter_context(tc.tile_pool(name="s2ps", bufs=1, space="PSUM"))
        if True:
            for rb in (range(RBLK) if 2 in stages else []):
                crow_o = s2p.tile([128, cD], f32, tag="crow_o")
                nc.gpsimd.indirect_dma_start(
                    out=crow_o[:],
                    out_offset=None,
                    in_=centers[:],
                    in_offset=bass.IndirectOffsetOnAxis(
                        ap=labo_sb[:, rb : rb + 1], axis=0
                    ),
                )
                xsb = s2p.tile([128, cD], f32, tag="xsb")
                nc.sync.dma_start(xsb[:], xs[rb * 128 : (rb + 1) * 128, :])

                # s_g partial: one-hot^T @ c over own rows (accumulated)
                orow_sb = s2p.tile([128, KO], f32, tag="orow")
                nc.sync.dma_start(orow_sb[:], orow[rb * 128 : (rb + 1) * 128, :])
                nc.tensor.matmul(
                    out=sg_ps[:],
                    lhsT=orow_sb[:],
                    rhs=crow_o[:],
                    start=(rb == 0),
                    stop=(rb == RBLK - 1),
                )

                # image loss partial: sum_d (x - c)^2 per own row
                diff = s2p.tile([128, cD], f32, tag="diff")
                nc.vector.tensor_tensor(
                    out=diff[:], in0=xsb[:], in1=crow_o[:], op=OP.subtract
                )
                sqv = s2p.tile([128, cD], f32, tag="sqv")
                nc.vector.tensor_tensor(
                    out=sqv[:], in0=diff[:], in1=diff[:], op=OP.mult
                )
                nc.vector.tensor_reduce(
                    out=imgacc[:, rb : rb + 1], in_=sqv[:], axis=AX.X, op=OP.add
                )

                # n for own rows -> free layout via [128,1] PE transpose
                csq = s2p.tile([128, cD], f32, tag="csq")
                nc.vector.tensor_tensor(
                    out=csq[:], in0=crow_o[:], in1=crow_o[:], op=OP.mult
                )
                ncol = s2p.tile([128, 1], f32, tag="ncol")
                nc.vector.tensor_reduce(
                    out=ncol[:], in_=csq[:], axis=AX.X, op=OP.add
                )
                ntp = s2ps.tile([1, 128], f32, space="PSUM", tag="tp2", name="ntp")
                nc.tensor.transpose(out=ntp[:], in_=ncol[:], identity=identity[:])
                nc.scalar.activation(
                    out=Le[64:65, rb * 128 : (rb + 1) * 128],
                    in_=ntp[0:1, :],
                    func=AF.Copy,
                    scale=1.0 / cD,
                    bias=1.0,
                )

                # Lc[rb][kc] = -2 c_own^T / D: transpose then scaled copy
                for kc in range(KC):
                    tp = s2ps.tile([128, 128], f32, space="PSUM", tag="tp2")
                    nc.tensor.transpose(
                        out=tp[:],
                        in_=crow_o[:, kc * 128 : (kc + 1) * 128],
                        identity=identity[:],
                    )
                    nc.scalar.activation(
                        out=Lc[rb][kc][:], in_=tp[:], func=AF.Copy, scale=-2.0 / cD
                    )

        # ---- fused column-chunk pipeline: gather -> transpose -> n -> pairwise
        inv_sqrt_d = 1.0 / float(np.sqrt(cD))
        with tc.tile_pool(name="f1", bufs=6) as s1p, tc.tile_pool(
            name="f1ps", bufs=2, space="PSUM"
        ) as s1ps, tc.tile_pool(name="f3", bufs=2) as s3p, tc.tile_pool(
            name="f3ps", bufs=1, space="PSUM"
        ) as s3ps, tc.tile_pool(name="f4", bufs=3) as s4p, tc.tile_pool(
            name="f4ps", bufs=2, space="PSUM"
        ) as s4ps:
            for ch in range(NCH):
                for c in range(512 // 128):
                    rc = ch * 4 + c
                    crow = s1p.tile([128, cD], f32, tag="crow")
                    nc.gpsimd.indirect_dma_start(
                        out=crow[:],
                        out_offset=None,
                        in_=centers[:],
                        in_offset=bass.IndirectOffsetOnAxis(
                            ap=lab_sb[:, rc : rc + 1], axis=0
                        ),
                    )
                    for kc in range(KC):
                        tp = s1ps.tile([128, 128], f32, space="PSUM", tag="tp")
                        nc.tensor.transpose(
                            out=tp[:],
                            in_=crow[:, kc * 128 : (kc + 1) * 128],
                            identity=identity[:],
                        )
                        nc.vector.tensor_copy(
                            out=A[kc][:, rc * 128 : (rc + 1) * 128], in_=tp[:]
                        )

                # n for this column chunk: ones^T @ (A/sqrt(D))^2
                np_t = s3ps.tile([1, 512], f32, space="PSUM", tag="npt")
                for kc in range(KC):
                    a2 = s3p.tile([128, 512], f32r, tag=f"a2_{kc}", name=f"a2_{kc}_{ch}")
                    nc.scalar.activation(
                        out=a2[:],
                        in_=A[kc][:, ch * 512 : (ch + 1) * 512],
                        func=AF.Square,
                        scale=inv_sqrt_d,
                    )
                    nc.tensor.matmul(
                        out=np_t[:],
                        lhsT=ones_col[:],
                        rhs=a2[:],
                        start=(kc == 0),
                        stop=(kc == KC - 1),
                    )
                nc.scalar.activation(
                    out=E[0:1, ch * 512 : (ch + 1) * 512], in_=np_t[:], func=AF.Copy
                )

                # pairwise: PSUM = 1 + pd + M*mask for (rb, ch); sum of 1/(.)
                for rb in range(RBLK):
                    pd_ps = s4ps.tile([128, 512], f32, space="PSUM", tag="pd")
                    for kc in range(KC):
                        nc.tensor.matmul(
                            out=pd_ps[:],
                            lhsT=Lc[rb][kc][:],
                            rhs=A[kc][:, ch * 512 : (ch + 1) * 512],
                            start=(kc == 0),
                            stop=False,
                        )
                    nc.tensor.matmul(
                        out=pd_ps[:],
                        lhsT=Le[:, rb * 128 : (rb + 1) * 128],
                        rhs=E[:, ch * 512 : (ch + 1) * 512],
                        start=False,
                        stop=True,
                    )
                    lnv = s4p.tile([128, 512], f32, tag="lnv")
                    nc.scalar.activation(out=lnv[:], in_=pd_ps[:], func=AF.Ln)
                    rdum = s4p.tile([128, 512], f32, tag="rdum")
                    nc.scalar.activation(
                        out=rdum[:],
                        in_=lnv[:],
                        func=AF.Exp,
                        scale=-1.0,
                        accum_out=racc[:, rb * NCH + ch : rb * NCH + ch + 1],
                    )

        # ---- S5: outputs
        if 2 in stages:
            sg_sb = constp.tile([KO, cD], f32, tag="sgsb")
            nc.vector.tensor_copy(out=sg_sb[:], in_=sg_ps[:])
            nc.sync.dma_start(sgout_d[:], sg_sb[:])
            nc.sync.dma_start(imgacc_d[:], imgacc[:])
        nc.sync.dma_start(racc_d[:], racc[:])
        nc.sync.dma_start(nout_d[:], E[0:1, :])

    import json as _json

    _orig_tjb = nc.to_json_bytes

    def _patched_tjb():
        m = _json.loads(_orig_tjb())
        _split_sync_waits(m)
        return _json.dumps(m).encode()

    nc.to_json_bytes = _patched_tjb
    return nc


def make_inputs(x, lab_i32, l2, cfg=None):
    """Host-side per-core input maps. l2 is int array [B] of labels_2."""
    cfg = dict(_DEFAULT_CFG if cfg is None else cfg)
    cB, cNG, cNC = cfg["B"], cfg["NG"], cfg["NCORES"]
    ROWS = cB // cNC
    KE = 128
    KO = 64
    SQM = 32768.0

    # E side: row0 = n/D (device), rows 1..NG = sqrt(M)*onehot, row64 = ones
    eb = np.zeros((KE, cB), np.float32)
    eb[64, :] = 1.0
    eb[1 + l2, np.arange(cB)] = SQM

    in_maps = []
    for k in range(cNC):
        sl = slice(k * ROWS, (k + 1) * ROWS)
        l2o = l2[sl]
        # L side: row0 = ones, rows 1..NG = sqrt(M)*onehot, row64 = n/D+1 (device)
        lbm = np.zeros((KE, ROWS), np.float32)
        lbm[0, :] = 1.0
        lbm[1 + l2o, np.arange(ROWS)] = SQM
        orow = np.zeros((ROWS, KO), np.float32)
        orow[np.arange(ROWS), l2o] = 1.0
        in_maps.append(
            {
                "lab": lab_i32,
                "onesv": np.ones((128, 1), np.float32),
                "idv": np.eye(128, dtype=np.float32),
                "labo": np.ascontiguousarray(lab_i32[sl]),
                "xs": np.ascontiguousarray(x[sl]),
                "eb": eb,
                "lb": lbm,
                "orow": orow,
            }
        )
    return in_maps


def combine(results, l2, yv, cfg=None):
    """Host-side combination of per-core partial outputs -> scalar loss."""
    cfg = dict(_DEFAULT_CFG if cfg is None else cfg)
    cB, cD, cNG = cfg["B"], cfg["D"], cfg["NG"]

    img = sum(r["imgacc"].astype(np.float64).sum() for r in results) / cB
    if yv == 1:
        return np.float32(img)

    n = results[0]["nout"][0].astype(np.float64) * cD  # nout holds n/D
    sg = sum(r["sgout"][:cNG].astype(np.float64) for r in results)
    inter_sum = sum(r["racc"].astype(np.float64).sum() for r in results)

    cnt = np.bincount(l2, minlength=cNG).astype(np.float64)
    nsum = np.bincount(l2, weights=n, minlength=cNG)
    n_same = float((cnt**2).sum())
    n_diff = float(cB * cB - n_same)
    intra_sum = float(((2.0 * cnt * nsum - 2.0 * (sg * sg).sum(axis=1)) / cD).sum())
    intra = intra_sum / max(n_same, 1.0)
    inter = float(inter_sum) / max(n_diff, 1.0)
    return np.float32(img + intra + inter)


def kernel(x, labels, labels_2, y, centers):
    global _last_results
    _import_concourse()
    from concourse.bass_utils import run_bass_kernel_spmd

    x = np.ascontiguousarray(np.asarray(x, dtype=np.float32))
    centers = np.ascontiguousarray(np.asarray(centers, dtype=np.float32))
    lab_i32 = np.ascontiguousarray(np.asarray(labels).astype(np.int32))
    l2 = np.asarray(labels_2).astype(np.int64)
    yv = int(np.asarray(y))

    if "prog" not in _cache:
        _cache["prog"] = build_program()
    nc = _cache["prog"]

    in_maps = make_inputs(x, lab_i32, l2)
    for m in in_maps:
        m["centers"] = centers

    res = run_bass_kernel_spmd(nc, in_maps, list(range(NCORES)))
    _last_results = res
    return combine(res.results, l2, yv)
